# revision 7
# baseline (speedup 1.0000x reference)
"""Edge-augmented multi-head graph attention on 8 TRN2 NeuronCores.

Math (per batch b=1, N=512 nodes, H=8 heads, D=64, NE=256, EE=128):
    q = nodes @ Wq + bq;  k,v = split(nodes @ Wkv + bkv);  e = edges @ We + be
    sim[h,i,j] = (q_h[i].(k_h[j]) + q_h[i].(e_h[i,j])) * D^-0.5
    attn = softmax_j(sim);  out[i] = (attn @ (v + e)) reshaped @ Wo + bo

Distribution: query rows i sharded 8-ways (64 rows/core). Softmax is over j
only, so cores are fully independent (no collectives).

Device algorithm avoids materializing e:
    sim2[i,j,h] = edges[i,j,:] . qe[i,h,:]   where qe[i,h] = We_h^T qhat_h[i]
    ae[i,h,:]   = sum_j attn[h,i,j] * edges[i,j,:]
    out2_h[i]   = ae[i,h] @ We_h
Zero-cost bias folds: be and bkv[v-half] add a constant vector to the inner
output -> folded into final_bias = (bv+be)@Wo + bo on host; bkv[k-half] and
the q.be term shift logits uniformly over j -> cancel in softmax; bq is
applied on device. Softmax computed without max subtraction (logits O(1))
and normalization deferred: Z accumulated via a ones-column of v.
"""

import sys

import numpy as np

if "/opt/trn_rl_repo" not in sys.path:
    sys.path.insert(0, "/opt/trn_rl_repo")

import ml_dtypes

B, N, NE, EE = 1, 512, 256, 128
H, D = 8, 64
INNER = H * D
NCORES = 8
IB = N // NCORES          # query rows per core
JT = N // 128             # j tiles
SCALE = float(D) ** -0.5

F32 = np.float32
BF16 = ml_dtypes.bfloat16

# dtype knobs (mybir names), resolved inside _build
EDGE_DT = "bfloat16"      # edges / qe / attn / v matmul dtype
KV_DT = "float32"         # k/v projection matmul dtype (float32 | float32r)

_PROG = None              # cached compiled Bass program


def _build():
    import concourse.bacc as bacc
    import concourse.tile as tile
    from concourse import mybir
    from concourse.masks import make_identity

    f32 = mybir.dt.float32
    edt = getattr(mybir.dt, EDGE_DT)
    kdt = getattr(mybir.dt, KV_DT)

    nc = bacc.Bacc("TRN2", target_bir_lowering=False, debug=False)

    # ---- DRAM I/O (per-core shapes; host pre-arranges layouts) ----
    d_nodes = nc.dram_tensor("nodes", [128, 4, NE], f32, kind="ExternalInput")
    d_qnodes = nc.dram_tensor("qnodes", [IB, NE], f32, kind="ExternalInput")
    d_egn = nc.dram_tensor("egn", [IB, 128, JT, EE], edt, kind="ExternalInput")
    d_egt = nc.dram_tensor("egt", [IB, EE, N], edt, kind="ExternalInput")
    d_wq = nc.dram_tensor("wq", [128, 2, INNER], f32, kind="ExternalInput")
    d_wkv = nc.dram_tensor("wkv", [128, 2, 2 * INNER], f32, kind="ExternalInput")
    d_we = nc.dram_tensor("we", [EE, INNER], f32, kind="ExternalInput")
    d_wo = nc.dram_tensor("wo", [128, 4, NE], f32, kind="ExternalInput")
    d_bq = nc.dram_tensor("bq_s", [D, H], f32, kind="ExternalInput")
    d_cb = nc.dram_tensor("cb", [128, 4], f32, kind="ExternalInput")
    d_fb = nc.dram_tensor("fb", [1, NE], f32, kind="ExternalInput")
    d_out = nc.dram_tensor("out", [IB, NE], f32, kind="ExternalOutput")

    with tile.TileContext(nc) as tc:
        with (
            tc.tile_pool(name="consts", bufs=1) as consts,
            tc.tile_pool(name="persist", bufs=1) as persist,
            tc.tile_pool(name="eg", bufs=8) as egp,
            tc.tile_pool(name="tmp", bufs=4) as tmp,
            tc.tile_pool(name="post", bufs=4) as postp,
        ):
            # ---------------- load constants ----------------
            nodes_sb = consts.tile([128, 4, NE], f32)
            nc.sync.dma_start(out=nodes_sb[:], in_=d_nodes[:])
            qn_sb = consts.tile([IB, NE], f32)
            nc.sync.dma_start(out=qn_sb[:], in_=d_qnodes[:])
            wq_sb = consts.tile([128, 2, INNER], f32)
            nc.sync.dma_start(out=wq_sb[:], in_=d_wq[:])
            wkv_sb = consts.tile([128, 2, 2 * INNER], f32)
            nc.sync.dma_start(out=wkv_sb[:], in_=d_wkv[:])
            we_sb = consts.tile([EE, INNER], f32)
            nc.sync.dma_start(out=we_sb[:], in_=d_we[:])
            wo_sb = consts.tile([128, 4, NE], f32)
            nc.sync.dma_start(out=wo_sb[:], in_=d_wo[:])
            bq_sb = consts.tile([D, H], f32)
            nc.sync.dma_start(out=bq_sb[:], in_=d_bq[:])
            cb_sb = consts.tile([128, 4], f32)
            nc.sync.dma_start(out=cb_sb[:], in_=d_cb[:])
            fb_sb = consts.tile([1, NE], f32)
            nc.sync.dma_start(out=fb_sb[:], in_=d_fb[:])

            ident = consts.tile([128, 128], f32)
            make_identity(nc, ident[:])
            ones1 = consts.tile([1, IB], f32)
            nc.vector.memset(ones1[:], 1.0)

            if KV_DT != "float32":
                wkv_c = consts.tile([128, 2, 2 * INNER], kdt)
                nc.vector.tensor_copy(wkv_c[:], wkv_sb[:])
            else:
                wkv_c = wkv_sb

            # ---------------- persistent intermediates ----------------
            nodesT = persist.tile([128, 2, N], f32)          # [ne%128, nechunk, n]
            qnT = persist.tile([128, 2, IB], f32)            # [ne%128, nechunk, i]
            kT = persist.tile([D, H, N], f32)                # [d, h, j]
            v_sb = persist.tile([128, JT, H, D + 1], edt)    # [j%128, jt, h, d|1]
            qT = persist.tile([D, H, IB], f32)               # [d, h, i]
            weT = persist.tile([D, H, EE], f32)              # [d, h, ee]
            qe_sb = persist.tile([EE, IB, H], edt)           # [ee, i, h]
            sim1 = persist.tile([128, JT, H, IB], f32)       # [j%128, jt, h, i]
            attnT = persist.tile([128, JT, H, IB], edt)      # [j%128, jt, h, i]
            ae_sb = persist.tile([EE, H, IB], f32)           # [ee, h, i]
            oi_sb = persist.tile([IB, H, D], f32)            # [i, h, d]
            oiT = persist.tile([128, 4, IB], f32)            # [inner%128, it, i]
            out_sb = persist.tile([IB, NE], f32)

            with (
                tc.tile_pool(name="psA", bufs=2, space="PSUM") as psA,
                tc.tile_pool(name="psB", bufs=2, space="PSUM") as psB,
            ):
                # nodesT: transpose nodes [n, ne] -> [ne, n]
                for t in range(4):
                    for c in range(2):
                        pt = psB.tile([128, 128], f32, tag="tr")
                        nc.tensor.transpose(
                            pt[:], nodes_sb[:, t, c * 128:(c + 1) * 128], ident[:]
                        )
                        nc.vector.tensor_copy(
                            nodesT[:, c, t * 128:(t + 1) * 128], pt[:]
                        )
                # qnT: transpose qnodes [i, ne] -> [ne, i]
                for c in range(2):
                    pt = psB.tile([128, IB], f32, tag="tr")
                    nc.tensor.transpose(
                        pt[:], qn_sb[:, c * 128:(c + 1) * 128], ident[0:IB, 0:IB]
                    )
                    nc.vector.tensor_copy(qnT[:, c, :], pt[:])
                # weT: transpose We [ee, inner] -> per head [d, ee]
                for h in range(H):
                    pt = psB.tile([D, EE], f32, tag="tr2")
                    nc.tensor.transpose(
                        pt[:], we_sb[:, h * D:(h + 1) * D], ident[:]
                    )
                    nc.vector.tensor_copy(weT[:, h, :], pt[:])

                if KV_DT == "float32":
                    nT_rhs = nodesT
                else:
                    nT_rhs = persist.tile([128, 2, N], kdt)
                    nc.vector.tensor_copy(nT_rhs[:], nodesT[:])

                for h in range(H):
                    pk = psA.tile([D, N], f32, tag="big")
                    for c in range(2):
                        nc.tensor.matmul(
                            pk[:],
                            wkv_c[:, c, h * D:(h + 1) * D],
                            nT_rhs[:, c, :],
                            start=(c == 0),
                            stop=(c == 1),
                        )
                    nc.vector.tensor_copy(kT[:, h, :], pk[:])

                # v[t] = nodes @ Wkv_v -> [j%128, inner] -> bf16 [jt,h,d]+ones
                for t in range(JT):
                    pv = psA.tile([128, N], f32, tag="big")
                    for c in range(2):
                        nc.tensor.matmul(
                            pv[:],
                            nT_rhs[:, c, t * 128:(t + 1) * 128],
                            wkv_c[:, c, INNER:2 * INNER],
                            start=(c == 0),
                            stop=(c == 1),
                        )
                    nc.vector.tensor_copy(v_sb[:, t, :, 0:D], pv[:])
                nc.vector.memset(v_sb[:, :, :, D:D + 1], 1.0)

                # qT[h] = scale * (Wq_h^T @ qnodesT + bq_h)
                for h in range(H):
                    pq = psB.tile([D, IB], f32, tag="trq")
                    for c in range(2):
                        nc.tensor.matmul(
                            pq[:],
                            wq_sb[:, c, h * D:(h + 1) * D],
                            qnT[:, c, :],
                            start=(c == 0),
                            stop=(c == 1),
                        )
                    nc.scalar.activation(
                        out=qT[:, h, :],
                        in_=pq[:],
                        func=mybir.ActivationFunctionType.Identity,
                        bias=bq_sb[:, h:h + 1],
                        scale=SCALE,
                    )

                # qe[h] = WeT_h @ qT_h -> [ee, i] (cast to edge dtype)
                for h in range(H):
                    pqe = psB.tile([EE, IB], f32, tag="trq")
                    nc.tensor.matmul(
                        pqe[:],
                        weT[:, h, :],
                        qT[:, h, :],
                        start=True,
                        stop=True,
                    )
                    nc.vector.tensor_copy(qe_sb[:, :, h], pqe[:])

                # sim1[jt,h] = kT_h[:, jt].T @ qT_h  (+ column mask bias)
                for h in range(H):
                    for jt in range(JT):
                        ps1 = psB.tile([128, IB], f32, tag="trq")
                        nc.tensor.matmul(
                            ps1[:],
                            kT[:, h, jt * 128:(jt + 1) * 128],
                            qT[:, h, :],
                            start=True,
                            stop=True,
                        )
                        nc.scalar.activation(
                            out=sim1[:, jt, h, :],
                            in_=ps1[:],
                            func=mybir.ActivationFunctionType.Identity,
                            bias=cb_sb[:, jt:jt + 1],
                            scale=1.0,
                        )

            # ---------------- main loop over own query rows ----------------
            with (
                tc.tile_pool(name="psS", bufs=3, space="PSUM") as psS,
                tc.tile_pool(name="psAE", bufs=3, space="PSUM") as psAE,
                tc.tile_pool(name="psO", bufs=2, space="PSUM") as psO,
            ):
                for i in range(IB):
                    egt = egp.tile([EE, N], edt, tag="egt")
                    nc.sync.dma_start(out=egt[:], in_=d_egt[i])
                    egn = egp.tile([128, JT, EE], edt, tag="egn")
                    nc.sync.dma_start(out=egn[:], in_=d_egn[i])

                    ps = psS.tile([128, JT, H], f32)
                    for jt in range(JT):
                        nc.tensor.matmul(
                            ps[:, jt, :],
                            egt[:, jt * 128:(jt + 1) * 128],
                            qe_sb[:, i, :],
                            start=True,
                            stop=True,
                        )
                    st = tmp.tile([128, JT, H], f32, tag="sim")
                    nc.vector.tensor_add(st[:], ps[:], sim1[:, :, :, i])
                    nc.scalar.activation(
                        out=attnT[:, :, :, i],
                        in_=st[:],
                        func=mybir.ActivationFunctionType.Exp,
                    )

                    pae = psAE.tile([EE, H], f32)
                    for jt in range(JT):
                        nc.tensor.matmul(
                            pae[:],
                            egn[:, jt, :],
                            attnT[:, jt, :, i],
                            start=(jt == 0),
                            stop=(jt == JT - 1),
                        )
                    nc.vector.tensor_copy(ae_sb[:, :, i], pae[:])

                # ---------------- epilogue ----------------
                for h in range(H):
                    po = psO.tile([IB, NE], f32, tag="po")
                    for jt in range(JT):
                        nc.tensor.matmul(
                            po[:, 0:D + 1],
                            attnT[:, jt, h, :],
                            v_sb[:, jt, h, :],
                            start=(jt == 0),
                            stop=False,
                            skip_group_check=True,
                        )
                    nc.tensor.matmul(
                        po[:, 0:D],
                        ae_sb[:, h, :],
                        we_sb[:, h * D:(h + 1) * D],
                        start=False,
                        stop=True,
                        skip_group_check=True,
                    )
                    rcp = postp.tile([IB, 1], f32, tag="rcp")
                    nc.vector.reciprocal(rcp[:], po[:, D:D + 1])
                    nc.vector.tensor_scalar_mul(oi_sb[:, h, :], po[:, 0:D], rcp[:])

                # transpose oi [i, inner] -> [inner, i]
                for it in range(4):
                    pt = psO.tile([128, IB], f32, tag="po")
                    nc.tensor.transpose(
                        pt[:],
                        oi_sb[:, it * 2:(it + 1) * 2, :].rearrange(
                            "p h d -> p (h d)"
                        ),
                        ident[0:IB, 0:IB],
                    )
                    nc.vector.tensor_copy(oiT[:, it, :], pt[:])

                # out = oi @ Wo + final_bias
                pf = psO.tile([IB, NE], f32, tag="po")
                for it in range(4):
                    nc.tensor.matmul(
                        pf[:],
                        oiT[:, it, :],
                        wo_sb[:, it, :],
                        start=(it == 0),
                        stop=False,
                        skip_group_check=True,
                    )
                nc.tensor.matmul(
                    pf[:],
                    ones1[:],
                    fb_sb[:],
                    start=False,
                    stop=True,
                    skip_group_check=True,
                )
                nc.vector.tensor_copy(out_sb[:], pf[:])
                nc.sync.dma_start(out=d_out[:], in_=out_sb[:])

    nc.compile()
    nc.finalize()
    return nc


def _get_prog():
    global _PROG
    if _PROG is None:
        _PROG = _build()
    return _PROG


def _prep_inputs(nodes, edges, mask, Wq, bq, Wkv, bkv, We, be, Wo, bo):
    """Host-side shard + layout prep. Returns list of 8 in_maps."""
    nodes = np.asarray(nodes, F32)[0]            # [N, NE]
    edges = np.asarray(edges, F32)[0]            # [N, N, EE]
    mask = np.asarray(mask)[0]                   # [N]
    Wq, bq = np.asarray(Wq, F32), np.asarray(bq, F32)
    Wkv, bkv = np.asarray(Wkv, F32), np.asarray(bkv, F32)
    We, be = np.asarray(We, F32), np.asarray(be, F32)
    Wo, bo = np.asarray(Wo, F32), np.asarray(bo, F32)

    e_np = BF16 if EDGE_DT == "bfloat16" else F32

    nodes_pre = np.ascontiguousarray(
        nodes.reshape(4, 128, NE).transpose(1, 0, 2))          # [128, 4, NE]
    wq_pre = np.ascontiguousarray(
        Wq.reshape(2, 128, INNER).transpose(1, 0, 2))          # [128, 2, INNER]
    wkv_pre = np.ascontiguousarray(
        Wkv.reshape(2, 128, 2 * INNER).transpose(1, 0, 2))
    wo_pre = np.ascontiguousarray(
        Wo.reshape(4, 128, NE).transpose(1, 0, 2))             # [128, 4, NE]
    bq_pre = np.ascontiguousarray(
        (bq * SCALE).reshape(H, D).T)                          # [D, H]
    cb = np.where(mask, 0.0, -1e30).astype(F32)
    cb_pre = np.ascontiguousarray(cb.reshape(4, 128).T)        # [128, 4]
    fb = ((bkv[INNER:] + be) @ Wo + bo).astype(F32)[None, :]   # [1, NE]

    common = dict(
        nodes=nodes_pre, wq=wq_pre, wkv=wkv_pre, we=We, wo=wo_pre,
        bq_s=bq_pre, cb=cb_pre, fb=fb,
    )
    in_maps = []
    for c in range(NCORES):
        sl = edges[c * IB:(c + 1) * IB]                        # [IB, N, EE]
        egn = np.ascontiguousarray(
            sl.reshape(IB, JT, 128, EE).transpose(0, 2, 1, 3)).astype(e_np)
        egt = np.ascontiguousarray(sl.transpose(0, 2, 1)).astype(e_np)
        qn = np.ascontiguousarray(nodes[c * IB:(c + 1) * IB])
        in_maps.append(dict(common, qnodes=qn, egn=egn, egt=egt))
    return in_maps


def kernel(**inputs):
    from concourse.bass_utils import run_bass_kernel_spmd

    nc = _get_prog()
    in_maps = _prep_inputs(**inputs)
    res = run_bass_kernel_spmd(nc, in_maps, core_ids=list(range(NCORES)))
    out = np.concatenate([res.results[c]["out"] for c in range(NCORES)], axis=0)
    return out.reshape(B, N, NE).astype(F32)


# revision 9
# speedup vs baseline: 1.5971x; 1.5971x over previous
"""Edge-augmented multi-head graph attention on 8 TRN2 NeuronCores.

Math (per batch b=1, N=512 nodes, H=8 heads, D=64, NE=256, EE=128):
    q = nodes @ Wq + bq;  k,v = split(nodes @ Wkv + bkv);  e = edges @ We + be
    sim[h,i,j] = (q_h[i].(k_h[j]) + q_h[i].(e_h[i,j])) * D^-0.5
    attn = softmax_j(sim);  out[i] = (attn @ (v + e)) reshaped @ Wo + bo

Distribution: query rows i sharded 8-ways (64 rows/core). Softmax is over j
only, so cores are fully independent (no collectives).

Device algorithm avoids materializing e:
    sim2[i,j,h] = edges[i,j,:] . qe[i,h,:]   where qe[i,h] = We_h^T qhat_h[i]
    ae[i,h,:]   = sum_j attn[h,i,j] * edges[i,j,:]
    out2_h[i]   = ae[i,h] @ We_h
Host supplies edges pre-cast to bf16 in both [i,j,ee] and [i,ee,j] layouts,
so no on-chip transposes of edge tiles are needed. Zero-cost bias folds:
be and bkv[v-half] add a constant vector to the inner output -> folded into
final_bias = (bv+be)@Wo + bo on host; bkv[k-half] and the q.be term shift
logits uniformly over j -> cancel in softmax; bq is applied on device.
Softmax computed without max subtraction (logits O(1)); normalization
deferred: Z accumulated via a ones-column appended to v. sim1 (q.k logits)
is accumulated into the sim2 PSUM tile via an identity-weight matmul, and
exp runs once per pair of rows straight out of PSUM.
"""

import sys

import numpy as np

if "/opt/trn_rl_repo" not in sys.path:
    sys.path.insert(0, "/opt/trn_rl_repo")

import ml_dtypes

B, N, NE, EE = 1, 512, 256, 128
H, D = 8, 64
INNER = H * D
NCORES = 8
IB = N // NCORES          # query rows per core
JT = N // 128             # j tiles
G = 8                     # query rows per edge-DMA group
SCALE = float(D) ** -0.5

F32 = np.float32
BF16 = ml_dtypes.bfloat16

_PROG = None              # cached compiled Bass program


def _build():
    import concourse.bacc as bacc
    import concourse.tile as tile
    from concourse import mybir
    from concourse.masks import make_identity

    f32 = mybir.dt.float32
    f32r = mybir.dt.float32r
    bf16 = mybir.dt.bfloat16
    AF = mybir.ActivationFunctionType

    nc = bacc.Bacc("TRN2", target_bir_lowering=False, debug=False)

    # ---- DRAM I/O (per-core shapes; host pre-arranges layouts) ----
    d_nodes = nc.dram_tensor("nodes", [128, 4, NE], f32, kind="ExternalInput")
    d_qnodes = nc.dram_tensor("qnodes", [IB, NE], f32, kind="ExternalInput")
    d_egn = nc.dram_tensor("egn", [IB, 128, JT, EE], bf16, kind="ExternalInput")
    d_egt = nc.dram_tensor("egt", [IB, EE, N], bf16, kind="ExternalInput")
    d_wq = nc.dram_tensor("wq", [128, 2, INNER], f32, kind="ExternalInput")
    d_wkv = nc.dram_tensor("wkv", [128, 2, 2 * INNER], f32r, kind="ExternalInput")
    d_we = nc.dram_tensor("we", [EE, INNER], f32, kind="ExternalInput")
    d_wo = nc.dram_tensor("wo", [128, 4, NE], f32r, kind="ExternalInput")
    d_bq = nc.dram_tensor("bq_s", [D, H], f32, kind="ExternalInput")
    d_cb = nc.dram_tensor("cb", [128, 4], f32, kind="ExternalInput")
    d_fb = nc.dram_tensor("fb", [1, NE], f32, kind="ExternalInput")
    d_out = nc.dram_tensor("out", [IB, NE], f32, kind="ExternalOutput")

    with tile.TileContext(nc) as tc:
        with (
            tc.tile_pool(name="consts", bufs=1) as consts,
            tc.tile_pool(name="persist", bufs=1) as persist,
            tc.tile_pool(name="eg", bufs=3) as egp,
            tc.tile_pool(name="post", bufs=4) as postp,
        ):
            # ---------------- load constants ----------------
            nodes_sb = consts.tile([128, 4, NE], f32)
            nc.sync.dma_start(out=nodes_sb[:], in_=d_nodes[:])
            qn_sb = consts.tile([IB, NE], f32)
            nc.sync.dma_start(out=qn_sb[:], in_=d_qnodes[:])
            wq_sb = consts.tile([128, 2, INNER], f32)
            nc.sync.dma_start(out=wq_sb[:], in_=d_wq[:])
            wkv_sb = consts.tile([128, 2, 2 * INNER], f32r)
            nc.sync.dma_start(out=wkv_sb[:], in_=d_wkv[:])
            we_sb = consts.tile([EE, INNER], f32)
            nc.sync.dma_start(out=we_sb[:], in_=d_we[:])
            wo_sb = consts.tile([128, 4, NE], f32r)
            nc.sync.dma_start(out=wo_sb[:], in_=d_wo[:])
            bq_sb = consts.tile([D, H], f32)
            nc.sync.dma_start(out=bq_sb[:], in_=d_bq[:])
            cb_sb = consts.tile([128, 4], f32)
            nc.sync.dma_start(out=cb_sb[:], in_=d_cb[:])
            fb_sb = consts.tile([1, NE], f32)
            nc.sync.dma_start(out=fb_sb[:], in_=d_fb[:])

            ident = consts.tile([128, 128], f32)
            make_identity(nc, ident[:])
            ident_bf = consts.tile([128, 128], bf16)
            make_identity(nc, ident_bf[:])
            ones1 = consts.tile([1, IB], f32)
            nc.vector.memset(ones1[:], 1.0)
            wq_bf = consts.tile([128, 2, INNER], bf16)
            nc.vector.tensor_copy(wq_bf[:], wq_sb[:])

            # ---------------- persistent intermediates ----------------
            nodesT = persist.tile([128, 2, N], f32r)         # [ne%128, c, n]
            qnT = persist.tile([128, 2, IB], bf16)           # [ne%128, c, i]
            kT = persist.tile([D, H, N], bf16)               # [d, h, j]
            v_sb = persist.tile([128, JT, H, D + 1], bf16)   # [j%128, jt, h, d|1]
            qT = persist.tile([D, H, IB], bf16)              # [d, h, i]
            weT = persist.tile([D, H, EE], bf16)             # [d, h, ee]
            qe_sb = persist.tile([EE, IB, H], bf16)          # [ee, i, h]
            sim1 = persist.tile([128, JT, H, IB], bf16)      # [j%128, jt, h, i]
            attnT = persist.tile([128, JT, H, IB], bf16)     # [j%128, jt, h, i]
            ae_sb = persist.tile([EE, H, IB], f32)           # [ee, h, i]
            oi_sb = persist.tile([IB, H, D], f32)            # [i, h, d]
            oiT = persist.tile([128, 4, IB], f32r)           # [inner%128, it, i]
            out_sb = persist.tile([IB, NE], f32)

            with (
                tc.tile_pool(name="psA", bufs=2, space="PSUM") as psA,
                tc.tile_pool(name="psB", bufs=2, space="PSUM") as psB,
            ):
                # nodesT: transpose nodes [n, ne] -> [ne, n]
                for t in range(4):
                    for c in range(2):
                        pt = psB.tile([128, 128], f32, tag="tr")
                        nc.tensor.transpose(
                            pt[:], nodes_sb[:, t, c * 128:(c + 1) * 128], ident[:]
                        )
                        nc.vector.tensor_copy(
                            nodesT[:, c, t * 128:(t + 1) * 128], pt[:]
                        )
                # qnT: transpose qnodes [i, ne] -> [ne, i] (cast bf16)
                for c in range(2):
                    pt = psB.tile([128, IB], f32, tag="tr")
                    nc.tensor.transpose(
                        pt[:], qn_sb[:, c * 128:(c + 1) * 128], ident[0:IB, 0:IB]
                    )
                    nc.vector.tensor_copy(qnT[:, c, :], pt[:])
                # weT: transpose We [ee, inner] -> per head [d, ee] (cast bf16)
                for h in range(H):
                    pt = psB.tile([D, EE], f32, tag="tr2")
                    nc.tensor.transpose(
                        pt[:], we_sb[:, h * D:(h + 1) * D], ident[:]
                    )
                    nc.vector.tensor_copy(weT[:, h, :], pt[:])

                # kT[h] = (Wkv_k^T @ nodesT) in f32r, stored bf16
                for h in range(H):
                    pk = psA.tile([D, N], f32, tag="big")
                    for c in range(2):
                        nc.tensor.matmul(
                            pk[:],
                            wkv_sb[:, c, h * D:(h + 1) * D],
                            nodesT[:, c, :],
                            start=(c == 0),
                            stop=(c == 1),
                        )
                    nc.vector.tensor_copy(kT[:, h, :], pk[:])

                # v[t] = nodes @ Wkv_v in f32r -> bf16 [jt,h,d] + ones col
                for t in range(JT):
                    pv = psA.tile([128, N], f32, tag="big")
                    for c in range(2):
                        nc.tensor.matmul(
                            pv[:],
                            nodesT[:, c, t * 128:(t + 1) * 128],
                            wkv_sb[:, c, INNER:2 * INNER],
                            start=(c == 0),
                            stop=(c == 1),
                        )
                    nc.vector.tensor_copy(v_sb[:, t, :, 0:D], pv[:])
                nc.vector.memset(v_sb[:, :, :, D:D + 1], 1.0)

                # qT[h] = scale * (Wq_h^T @ qnodesT + bq_h)  (bf16 matmul)
                for h in range(H):
                    pq = psB.tile([D, IB], f32, tag="trq")
                    for c in range(2):
                        nc.tensor.matmul(
                            pq[:],
                            wq_bf[:, c, h * D:(h + 1) * D],
                            qnT[:, c, :],
                            start=(c == 0),
                            stop=(c == 1),
                        )
                    nc.scalar.activation(
                        out=qT[:, h, :],
                        in_=pq[:],
                        func=AF.Identity,
                        bias=bq_sb[:, h:h + 1],
                        scale=SCALE,
                    )

                # qe[h] = WeT_h @ qT_h -> [ee, i]
                for h in range(H):
                    pqe = psB.tile([EE, IB], f32, tag="trq")
                    nc.tensor.matmul(
                        pqe[:], weT[:, h, :], qT[:, h, :], start=True, stop=True
                    )
                    nc.vector.tensor_copy(qe_sb[:, :, h], pqe[:])

                # sim1[jt,h] = kT_h[:, jt].T @ qT_h  (+ column mask bias)
                for h in range(H):
                    for jt in range(JT):
                        ps1 = psB.tile([128, IB], f32, tag="trq")
                        nc.tensor.matmul(
                            ps1[:],
                            kT[:, h, jt * 128:(jt + 1) * 128],
                            qT[:, h, :],
                            start=True,
                            stop=True,
                        )
                        nc.vector.tensor_scalar(
                            out=sim1[:, jt, h, :],
                            in0=ps1[:],
                            scalar1=cb_sb[:, jt:jt + 1],
                            scalar2=None,
                            op0=mybir.AluOpType.add,
                        )

            # ---------------- main loop over own query rows ----------------
            with (
                tc.tile_pool(name="psS", bufs=3, space="PSUM") as psS,
                tc.tile_pool(name="psAE", bufs=3, space="PSUM") as psAE,
                tc.tile_pool(name="psO", bufs=2, space="PSUM") as psO,
            ):
                egts, egns = [], []
                for g in range(IB // G):
                    egt = egp.tile([EE, G, N], bf16, tag="egt")
                    nc.sync.dma_start(
                        out=egt[:],
                        in_=d_egt[g * G:(g + 1) * G].rearrange("g p j -> p g j"),
                    )
                    egn = egp.tile([128, G, JT, EE], bf16, tag="egn")
                    nc.gpsimd.dma_start(
                        out=egn[:],
                        in_=d_egn[g * G:(g + 1) * G].rearrange(
                            "g p t e -> p g t e"
                        ),
                    )
                    egts.append(egt)
                    egns.append(egn)

                def sim_block(i, ps, u):
                    """sim1 copy + 4 sim2 matmuls into ps[:, u] for row i."""
                    gi, go = divmod(i, G)
                    nc.tensor.matmul(
                        ps[:, u, :, :],
                        ident_bf[:],
                        sim1[:, :, :, i],
                        start=True,
                        stop=False,
                        skip_group_check=True,
                    )
                    for jt in range(JT):
                        nc.tensor.matmul(
                            ps[:, u, jt, :],
                            egts[gi][:, go, jt * 128:(jt + 1) * 128],
                            qe_sb[:, i, :],
                            start=False,
                            stop=(jt == JT - 1),
                            skip_group_check=True,
                        )

                def ae_block(i):
                    gi, go = divmod(i, G)
                    pae = psAE.tile([EE, H], f32, tag="ae")
                    for jt in range(JT):
                        nc.tensor.matmul(
                            pae[:],
                            egns[gi][:, go, jt, :],
                            attnT[:, jt, :, i],
                            start=(jt == 0),
                            stop=(jt == JT - 1),
                        )
                    nc.vector.tensor_copy(ae_sb[:, :, i], pae[:])

                prev = None
                for p in range(IB // 2):
                    i0 = 2 * p
                    ps = psS.tile([128, 2, JT, H], f32, tag="sim")
                    sim_block(i0, ps, 0)
                    sim_block(i0 + 1, ps, 1)
                    nc.scalar.activation(
                        out=attnT[:, :, :, i0:i0 + 2].rearrange(
                            "p t h i -> p i t h"
                        ),
                        in_=ps[:],
                        func=AF.Exp,
                    )
                    if prev is not None:
                        ae_block(prev)
                        ae_block(prev + 1)
                    prev = i0
                ae_block(prev)
                ae_block(prev + 1)

                # ---------------- epilogue ----------------
                for h in range(H):
                    po = psO.tile([IB, NE], f32, tag="po")
                    for jt in range(JT):
                        nc.tensor.matmul(
                            po[:, 0:D + 1],
                            attnT[:, jt, h, :],
                            v_sb[:, jt, h, :],
                            start=(jt == 0),
                            stop=False,
                            skip_group_check=True,
                        )
                    nc.tensor.matmul(
                        po[:, 0:D],
                        ae_sb[:, h, :],
                        we_sb[:, h * D:(h + 1) * D],
                        start=False,
                        stop=True,
                        skip_group_check=True,
                    )
                    rcp = postp.tile([IB, 1], f32, tag="rcp")
                    nc.vector.reciprocal(rcp[:], po[:, D:D + 1])
                    nc.vector.tensor_scalar_mul(oi_sb[:, h, :], po[:, 0:D], rcp[:])

                # transpose oi [i, inner] -> [inner, i]
                for it in range(4):
                    pt = psO.tile([128, IB], f32, tag="po")
                    nc.tensor.transpose(
                        pt[:],
                        oi_sb[:, it * 2:(it + 1) * 2, :],
                        ident[0:IB, 0:IB],
                    )
                    nc.vector.tensor_copy(oiT[:, it, :], pt[:])

                # out = oi @ Wo + final_bias
                pf = psO.tile([IB, NE], f32, tag="po")
                for it in range(4):
                    nc.tensor.matmul(
                        pf[:],
                        oiT[:, it, :],
                        wo_sb[:, it, :],
                        start=(it == 0),
                        stop=False,
                        skip_group_check=True,
                    )
                nc.tensor.matmul(
                    pf[:],
                    ones1[:],
                    fb_sb[:],
                    start=False,
                    stop=True,
                    skip_group_check=True,
                )
                nc.vector.tensor_copy(out_sb[:], pf[:])
                nc.sync.dma_start(out=d_out[:], in_=out_sb[:])

    nc.compile()
    nc.finalize()
    return nc


def _get_prog():
    global _PROG
    if _PROG is None:
        _PROG = _build()
    return _PROG


def _prep_inputs(nodes, edges, mask, Wq, bq, Wkv, bkv, We, be, Wo, bo):
    """Host-side shard + layout prep. Returns list of 8 in_maps."""
    nodes = np.asarray(nodes, F32)[0]            # [N, NE]
    edges = np.asarray(edges, F32)[0]            # [N, N, EE]
    mask = np.asarray(mask)[0]                   # [N]
    Wq, bq = np.asarray(Wq, F32), np.asarray(bq, F32)
    Wkv, bkv = np.asarray(Wkv, F32), np.asarray(bkv, F32)
    We, be = np.asarray(We, F32), np.asarray(be, F32)
    Wo, bo = np.asarray(Wo, F32), np.asarray(bo, F32)

    nodes_pre = np.ascontiguousarray(
        nodes.reshape(4, 128, NE).transpose(1, 0, 2))          # [128, 4, NE]
    wq_pre = np.ascontiguousarray(
        Wq.reshape(2, 128, INNER).transpose(1, 0, 2))          # [128, 2, INNER]
    wkv_pre = np.ascontiguousarray(
        Wkv.reshape(2, 128, 2 * INNER).transpose(1, 0, 2))
    wo_pre = np.ascontiguousarray(
        Wo.reshape(4, 128, NE).transpose(1, 0, 2))             # [128, 4, NE]
    bq_pre = np.ascontiguousarray(
        (bq * SCALE).reshape(H, D).T)                          # [D, H]
    cb = np.where(mask, 0.0, -1e30).astype(F32)
    cb_pre = np.ascontiguousarray(cb.reshape(4, 128).T)        # [128, 4]
    fb = ((bkv[INNER:] + be) @ Wo + bo).astype(F32)[None, :]   # [1, NE]

    common = dict(
        nodes=nodes_pre, wq=wq_pre, wkv=wkv_pre, we=We, wo=wo_pre,
        bq_s=bq_pre, cb=cb_pre, fb=fb,
    )
    in_maps = []
    for c in range(NCORES):
        sl = edges[c * IB:(c + 1) * IB]                        # [IB, N, EE]
        egn = np.ascontiguousarray(
            sl.reshape(IB, JT, 128, EE).transpose(0, 2, 1, 3)).astype(BF16)
        egt = np.ascontiguousarray(sl.transpose(0, 2, 1)).astype(BF16)
        qn = np.ascontiguousarray(nodes[c * IB:(c + 1) * IB])
        in_maps.append(dict(common, qnodes=qn, egn=egn, egt=egt))
    return in_maps


def kernel(**inputs):
    from concourse.bass_utils import run_bass_kernel_spmd

    nc = _get_prog()
    in_maps = _prep_inputs(**inputs)
    res = run_bass_kernel_spmd(nc, in_maps, core_ids=list(range(NCORES)))
    out = np.concatenate([res.results[c]["out"] for c in range(NCORES)], axis=0)
    return out.reshape(B, N, NE).astype(F32)


# revision 10
# speedup vs baseline: 1.6730x; 1.0475x over previous
"""Edge-augmented multi-head graph attention on 8 TRN2 NeuronCores.

Math (per batch b=1, N=512 nodes, H=8 heads, D=64, NE=256, EE=128):
    q = nodes @ Wq + bq;  k,v = split(nodes @ Wkv + bkv);  e = edges @ We + be
    sim[h,i,j] = (q_h[i].(k_h[j]) + q_h[i].(e_h[i,j])) * D^-0.5
    attn = softmax_j(sim);  out[i] = (attn @ (v + e)) reshaped @ Wo + bo

Distribution: query rows i sharded 8-ways (64 rows/core). Softmax is over j
only, so cores are fully independent (no collectives).

Device algorithm avoids materializing e:
    sim2[i,j,h] = edges[i,j,:] . qe[i,h,:]   where qe[i,h] = We_h^T qhat_h[i]
    ae[i,h,:]   = sum_j attn[h,i,j] * edges[i,j,:]
    out2_h[i]   = ae[i,h] @ We_h
Host supplies edges pre-cast to bf16 in both [i,j,ee] and [i,ee,j] layouts,
so no on-chip transposes of edge tiles are needed. Zero-cost bias folds:
be and bkv[v-half] add a constant vector to the inner output -> folded into
final_bias = (bv+be)@Wo + bo on host; bkv[k-half] and the q.be term shift
logits uniformly over j -> cancel in softmax; bq is applied on device.
Softmax computed without max subtraction (logits O(1)); normalization
deferred: Z accumulated via a ones-column appended to v. sim1 (q.k logits)
is accumulated into the sim2 PSUM tile via an identity-weight matmul, and
exp runs once per pair of rows straight out of PSUM.
"""

import sys

import numpy as np

if "/opt/trn_rl_repo" not in sys.path:
    sys.path.insert(0, "/opt/trn_rl_repo")

import ml_dtypes

B, N, NE, EE = 1, 512, 256, 128
H, D = 8, 64
INNER = H * D
NCORES = 8
IB = N // NCORES          # query rows per core
JT = N // 128             # j tiles
G = 8                     # query rows per edge-DMA group
SCALE = float(D) ** -0.5

F32 = np.float32
BF16 = ml_dtypes.bfloat16

_PROG = None              # cached compiled Bass program


def _build():
    import concourse.bacc as bacc
    import concourse.tile as tile
    from concourse import mybir
    from concourse.masks import make_identity

    f32 = mybir.dt.float32
    f32r = mybir.dt.float32r
    bf16 = mybir.dt.bfloat16
    AF = mybir.ActivationFunctionType

    nc = bacc.Bacc("TRN2", target_bir_lowering=False, debug=False)

    # ---- DRAM I/O (per-core shapes; host pre-arranges layouts) ----
    d_nodes = nc.dram_tensor("nodes", [128, 4, NE], f32, kind="ExternalInput")
    d_qnodes = nc.dram_tensor("qnodes", [IB, NE], f32, kind="ExternalInput")
    d_egn = nc.dram_tensor("egn", [IB, 128, JT, EE], bf16, kind="ExternalInput")
    d_egt = nc.dram_tensor("egt", [IB, EE, N], bf16, kind="ExternalInput")
    d_wq = nc.dram_tensor("wq", [128, 2, INNER], f32, kind="ExternalInput")
    d_wkv = nc.dram_tensor("wkv", [128, 2, 2 * INNER], f32r, kind="ExternalInput")
    d_we = nc.dram_tensor("we", [EE, INNER], f32, kind="ExternalInput")
    d_wo = nc.dram_tensor("wo", [128, 4, NE], f32r, kind="ExternalInput")
    d_bq = nc.dram_tensor("bq_s", [D, H], f32, kind="ExternalInput")
    d_cb = nc.dram_tensor("cb", [128, 4], f32, kind="ExternalInput")
    d_fb = nc.dram_tensor("fb", [1, NE], f32, kind="ExternalInput")
    d_out = nc.dram_tensor("out", [IB, NE], f32, kind="ExternalOutput")

    with tile.TileContext(nc) as tc:
        with (
            tc.tile_pool(name="consts", bufs=1) as consts,
            tc.tile_pool(name="persist", bufs=1) as persist,
            tc.tile_pool(name="eg", bufs=5) as egp,
            tc.tile_pool(name="post", bufs=4) as postp,
        ):
            # ---------------- load constants (SWDGE queue; HWDGE is
            # reserved for the big edge streams) ----------------
            wkv_sb = consts.tile([128, 2, 2 * INNER], f32r)
            nc.gpsimd.dma_start(out=wkv_sb[:], in_=d_wkv[:])
            nodes_sb = consts.tile([128, 4, NE], f32)
            nc.gpsimd.dma_start(out=nodes_sb[:], in_=d_nodes[:])
            qn_sb = consts.tile([IB, NE], f32)
            nc.gpsimd.dma_start(out=qn_sb[:], in_=d_qnodes[:])
            wq_sb = consts.tile([128, 2, INNER], f32)
            nc.gpsimd.dma_start(out=wq_sb[:], in_=d_wq[:])
            we_sb = consts.tile([EE, INNER], f32)
            nc.gpsimd.dma_start(out=we_sb[:], in_=d_we[:])
            wo_sb = consts.tile([128, 4, NE], f32r)
            nc.gpsimd.dma_start(out=wo_sb[:], in_=d_wo[:])
            bq_sb = consts.tile([D, H], f32)
            nc.gpsimd.dma_start(out=bq_sb[:], in_=d_bq[:])
            cb_sb = consts.tile([128, 4], f32)
            nc.gpsimd.dma_start(out=cb_sb[:], in_=d_cb[:])
            fb_sb = consts.tile([1, NE], f32)
            nc.gpsimd.dma_start(out=fb_sb[:], in_=d_fb[:])

            # edge streams: two HWDGE queues, issued up front
            egts, egns = [], []
            for g in range(IB // G):
                egt = egp.tile([EE, G, N], bf16, tag="egt")
                nc.sync.dma_start(
                    out=egt[:],
                    in_=d_egt[g * G:(g + 1) * G].rearrange("g p j -> p g j"),
                )
                egn = egp.tile([128, G, JT, EE], bf16, tag="egn")
                nc.scalar.dma_start(
                    out=egn[:],
                    in_=d_egn[g * G:(g + 1) * G].rearrange(
                        "g p t e -> p g t e"
                    ),
                )
                egts.append(egt)
                egns.append(egn)

            ident = consts.tile([128, 128], f32)
            make_identity(nc, ident[:])
            ident_bf = consts.tile([128, 128], bf16)
            make_identity(nc, ident_bf[:])
            ones1 = consts.tile([1, IB], f32)
            nc.vector.memset(ones1[:], 1.0)
            wq_bf = consts.tile([128, 2, INNER], bf16)
            nc.vector.tensor_copy(wq_bf[:], wq_sb[:])

            # ---------------- persistent intermediates ----------------
            nodesT = persist.tile([128, 2, N], f32r)         # [ne%128, c, n]
            qnT = persist.tile([128, 2, IB], bf16)           # [ne%128, c, i]
            kT = persist.tile([D, H, N], bf16)               # [d, h, j]
            v_sb = persist.tile([128, JT, H, D + 1], bf16)   # [j%128, jt, h, d|1]
            qT = persist.tile([D, H, IB], bf16)              # [d, h, i]
            weT = persist.tile([D, H, EE], bf16)             # [d, h, ee]
            qe_sb = persist.tile([EE, IB, H], bf16)          # [ee, i, h]
            sim1 = persist.tile([128, JT, H, IB], bf16)      # [j%128, jt, h, i]
            attnT = persist.tile([128, JT, H, IB], bf16)     # [j%128, jt, h, i]
            ae_sb = persist.tile([EE, H, IB], f32)           # [ee, h, i]
            oi_sb = persist.tile([IB, H, D], f32)            # [i, h, d]
            oiT = persist.tile([128, 4, IB], f32r)           # [inner%128, it, i]
            out_sb = persist.tile([IB, NE], f32)

            with (
                tc.tile_pool(name="psA", bufs=2, space="PSUM") as psA,
                tc.tile_pool(name="psB", bufs=2, space="PSUM") as psB,
            ):
                # nodesT: transpose nodes [n, ne] -> [ne, n]
                for t in range(4):
                    for c in range(2):
                        pt = psB.tile([128, 128], f32, tag="tr")
                        nc.tensor.transpose(
                            pt[:], nodes_sb[:, t, c * 128:(c + 1) * 128], ident[:]
                        )
                        nc.vector.tensor_copy(
                            nodesT[:, c, t * 128:(t + 1) * 128], pt[:]
                        )
                # qnT: transpose qnodes [i, ne] -> [ne, i] (cast bf16)
                for c in range(2):
                    pt = psB.tile([128, IB], f32, tag="tr")
                    nc.tensor.transpose(
                        pt[:], qn_sb[:, c * 128:(c + 1) * 128], ident[0:IB, 0:IB]
                    )
                    nc.vector.tensor_copy(qnT[:, c, :], pt[:])
                # weT: transpose We [ee, inner] -> per head [d, ee] (cast bf16)
                for h in range(H):
                    pt = psB.tile([D, EE], f32, tag="tr2")
                    nc.tensor.transpose(
                        pt[:], we_sb[:, h * D:(h + 1) * D], ident[:]
                    )
                    nc.vector.tensor_copy(weT[:, h, :], pt[:])

                # kT[h] = (Wkv_k^T @ nodesT) in f32r, stored bf16
                for h in range(H):
                    pk = psA.tile([D, N], f32, tag="big")
                    for c in range(2):
                        nc.tensor.matmul(
                            pk[:],
                            wkv_sb[:, c, h * D:(h + 1) * D],
                            nodesT[:, c, :],
                            start=(c == 0),
                            stop=(c == 1),
                        )
                    nc.vector.tensor_copy(kT[:, h, :], pk[:])

                # v[t] = nodes @ Wkv_v in f32r -> bf16 [jt,h,d] + ones col
                for t in range(JT):
                    pv = psA.tile([128, N], f32, tag="big")
                    for c in range(2):
                        nc.tensor.matmul(
                            pv[:],
                            nodesT[:, c, t * 128:(t + 1) * 128],
                            wkv_sb[:, c, INNER:2 * INNER],
                            start=(c == 0),
                            stop=(c == 1),
                        )
                    nc.vector.tensor_copy(v_sb[:, t, :, 0:D], pv[:])
                nc.vector.memset(v_sb[:, :, :, D:D + 1], 1.0)

                # qT[h] = scale * (Wq_h^T @ qnodesT + bq_h)  (bf16 matmul)
                for h in range(H):
                    pq = psB.tile([D, IB], f32, tag="trq")
                    for c in range(2):
                        nc.tensor.matmul(
                            pq[:],
                            wq_bf[:, c, h * D:(h + 1) * D],
                            qnT[:, c, :],
                            start=(c == 0),
                            stop=(c == 1),
                        )
                    nc.scalar.activation(
                        out=qT[:, h, :],
                        in_=pq[:],
                        func=AF.Identity,
                        bias=bq_sb[:, h:h + 1],
                        scale=SCALE,
                    )

                # qe[h] = WeT_h @ qT_h -> [ee, i]
                for h in range(H):
                    pqe = psB.tile([EE, IB], f32, tag="trq")
                    nc.tensor.matmul(
                        pqe[:], weT[:, h, :], qT[:, h, :], start=True, stop=True
                    )
                    nc.vector.tensor_copy(qe_sb[:, :, h], pqe[:])

                # sim1[jt,h] = kT_h[:, jt].T @ qT_h  (+ column mask bias)
                for h in range(H):
                    for jt in range(JT):
                        ps1 = psB.tile([128, IB], f32, tag="trq")
                        nc.tensor.matmul(
                            ps1[:],
                            kT[:, h, jt * 128:(jt + 1) * 128],
                            qT[:, h, :],
                            start=True,
                            stop=True,
                        )
                        nc.vector.tensor_scalar(
                            out=sim1[:, jt, h, :],
                            in0=ps1[:],
                            scalar1=cb_sb[:, jt:jt + 1],
                            scalar2=None,
                            op0=mybir.AluOpType.add,
                        )

            # ---------------- main loop over own query rows ----------------
            with (
                tc.tile_pool(name="psS", bufs=3, space="PSUM") as psS,
                tc.tile_pool(name="psAE", bufs=3, space="PSUM") as psAE,
                tc.tile_pool(name="psO", bufs=2, space="PSUM") as psO,
            ):
                def sim_block(i, ps, u):
                    """sim1 copy + 4 sim2 matmuls into ps[:, u] for row i."""
                    gi, go = divmod(i, G)
                    nc.tensor.matmul(
                        ps[:, u, :, :],
                        ident_bf[:],
                        sim1[:, :, :, i],
                        start=True,
                        stop=False,
                        skip_group_check=True,
                    )
                    for jt in range(JT):
                        nc.tensor.matmul(
                            ps[:, u, jt, :],
                            egts[gi][:, go, jt * 128:(jt + 1) * 128],
                            qe_sb[:, i, :],
                            start=False,
                            stop=(jt == JT - 1),
                            skip_group_check=True,
                        )

                def ae_block(i):
                    gi, go = divmod(i, G)
                    pae = psAE.tile([EE, H], f32, tag="ae")
                    for jt in range(JT):
                        nc.tensor.matmul(
                            pae[:],
                            egns[gi][:, go, jt, :],
                            attnT[:, jt, :, i],
                            start=(jt == 0),
                            stop=(jt == JT - 1),
                        )
                    nc.vector.tensor_copy(ae_sb[:, :, i], pae[:])

                prev = None
                for p in range(IB // 2):
                    i0 = 2 * p
                    ps = psS.tile([128, 2, JT, H], f32, tag="sim")
                    sim_block(i0, ps, 0)
                    sim_block(i0 + 1, ps, 1)
                    nc.scalar.activation(
                        out=attnT[:, :, :, i0:i0 + 2].rearrange(
                            "p t h i -> p i t h"
                        ),
                        in_=ps[:],
                        func=AF.Exp,
                    )
                    if prev is not None:
                        ae_block(prev)
                        ae_block(prev + 1)
                    prev = i0
                ae_block(prev)
                ae_block(prev + 1)

                # ---------------- epilogue ----------------
                for h in range(H):
                    po = psO.tile([IB, NE], f32, tag="po")
                    for jt in range(JT):
                        nc.tensor.matmul(
                            po[:, 0:D + 1],
                            attnT[:, jt, h, :],
                            v_sb[:, jt, h, :],
                            start=(jt == 0),
                            stop=False,
                            skip_group_check=True,
                        )
                    nc.tensor.matmul(
                        po[:, 0:D],
                        ae_sb[:, h, :],
                        we_sb[:, h * D:(h + 1) * D],
                        start=False,
                        stop=True,
                        skip_group_check=True,
                    )
                    rcp = postp.tile([IB, 1], f32, tag="rcp")
                    nc.vector.reciprocal(rcp[:], po[:, D:D + 1])
                    nc.vector.tensor_scalar_mul(oi_sb[:, h, :], po[:, 0:D], rcp[:])

                # transpose oi [i, inner] -> [inner, i]
                for it in range(4):
                    pt = psO.tile([128, IB], f32, tag="po")
                    nc.tensor.transpose(
                        pt[:],
                        oi_sb[:, it * 2:(it + 1) * 2, :],
                        ident[0:IB, 0:IB],
                    )
                    nc.vector.tensor_copy(oiT[:, it, :], pt[:])

                # out = oi @ Wo + final_bias
                pf = psO.tile([IB, NE], f32, tag="po")
                for it in range(4):
                    nc.tensor.matmul(
                        pf[:],
                        oiT[:, it, :],
                        wo_sb[:, it, :],
                        start=(it == 0),
                        stop=False,
                        skip_group_check=True,
                    )
                nc.tensor.matmul(
                    pf[:],
                    ones1[:],
                    fb_sb[:],
                    start=False,
                    stop=True,
                    skip_group_check=True,
                )
                nc.vector.tensor_copy(out_sb[:], pf[:])
                nc.gpsimd.dma_start(out=d_out[:], in_=out_sb[:])

    nc.compile()
    nc.finalize()
    return nc


def _get_prog():
    global _PROG
    if _PROG is None:
        _PROG = _build()
    return _PROG


def _prep_inputs(nodes, edges, mask, Wq, bq, Wkv, bkv, We, be, Wo, bo):
    """Host-side shard + layout prep. Returns list of 8 in_maps."""
    nodes = np.asarray(nodes, F32)[0]            # [N, NE]
    edges = np.asarray(edges, F32)[0]            # [N, N, EE]
    mask = np.asarray(mask)[0]                   # [N]
    Wq, bq = np.asarray(Wq, F32), np.asarray(bq, F32)
    Wkv, bkv = np.asarray(Wkv, F32), np.asarray(bkv, F32)
    We, be = np.asarray(We, F32), np.asarray(be, F32)
    Wo, bo = np.asarray(Wo, F32), np.asarray(bo, F32)

    nodes_pre = np.ascontiguousarray(
        nodes.reshape(4, 128, NE).transpose(1, 0, 2))          # [128, 4, NE]
    wq_pre = np.ascontiguousarray(
        Wq.reshape(2, 128, INNER).transpose(1, 0, 2))          # [128, 2, INNER]
    wkv_pre = np.ascontiguousarray(
        Wkv.reshape(2, 128, 2 * INNER).transpose(1, 0, 2))
    wo_pre = np.ascontiguousarray(
        Wo.reshape(4, 128, NE).transpose(1, 0, 2))             # [128, 4, NE]
    bq_pre = np.ascontiguousarray(
        (bq * SCALE).reshape(H, D).T)                          # [D, H]
    cb = np.where(mask, 0.0, -1e30).astype(F32)
    cb_pre = np.ascontiguousarray(cb.reshape(4, 128).T)        # [128, 4]
    fb = ((bkv[INNER:] + be) @ Wo + bo).astype(F32)[None, :]   # [1, NE]

    common = dict(
        nodes=nodes_pre, wq=wq_pre, wkv=wkv_pre, we=We, wo=wo_pre,
        bq_s=bq_pre, cb=cb_pre, fb=fb,
    )
    in_maps = []
    for c in range(NCORES):
        sl = edges[c * IB:(c + 1) * IB]                        # [IB, N, EE]
        egn = np.ascontiguousarray(
            sl.reshape(IB, JT, 128, EE).transpose(0, 2, 1, 3)).astype(BF16)
        egt = np.ascontiguousarray(sl.transpose(0, 2, 1)).astype(BF16)
        qn = np.ascontiguousarray(nodes[c * IB:(c + 1) * IB])
        in_maps.append(dict(common, qnodes=qn, egn=egn, egt=egt))
    return in_maps


def kernel(**inputs):
    from concourse.bass_utils import run_bass_kernel_spmd

    nc = _get_prog()
    in_maps = _prep_inputs(**inputs)
    res = run_bass_kernel_spmd(nc, in_maps, core_ids=list(range(NCORES)))
    out = np.concatenate([res.results[c]["out"] for c in range(NCORES)], axis=0)
    return out.reshape(B, N, NE).astype(F32)


# revision 12
# speedup vs baseline: 1.6942x; 1.0127x over previous
"""Edge-augmented multi-head graph attention on 8 TRN2 NeuronCores.

Math (per batch b=1, N=512 nodes, H=8 heads, D=64, NE=256, EE=128):
    q = nodes @ Wq + bq;  k,v = split(nodes @ Wkv + bkv);  e = edges @ We + be
    sim[h,i,j] = (q_h[i].(k_h[j]) + q_h[i].(e_h[i,j])) * D^-0.5
    attn = softmax_j(sim);  out[i] = (attn @ (v + e)) reshaped @ Wo + bo

Distribution: query rows i sharded 8-ways (64 rows/core). Softmax is over j
only, so cores are fully independent (no collectives).

Device algorithm avoids materializing e:
    sim2[i,j,h] = edges[i,j,:] . qe[i,h,:]   where qe[i,h] = We_h^T qhat_h[i]
    ae[i,h,:]   = sum_j attn[h,i,j] * edges[i,j,:]
    out2_h[i]   = ae[i,h] @ We_h
Host supplies edges pre-cast to bf16 in both [i,j,ee] and [i,ee,j] layouts,
so no on-chip transposes of edge tiles are needed. Zero-cost bias folds:
be and bkv[v-half] add a constant vector to the inner output -> folded into
final_bias = (bv+be)@Wo + bo on host; bkv[k-half] and the q.be term shift
logits uniformly over j -> cancel in softmax; bq is applied on device.
Softmax computed without max subtraction (logits O(1)); normalization
deferred: Z accumulated via a ones-column appended to v. sim1 (q.k logits)
is accumulated into the sim2 PSUM tile via an identity-weight matmul, and
exp runs once per pair of rows straight out of PSUM.
"""

import sys

import numpy as np

if "/opt/trn_rl_repo" not in sys.path:
    sys.path.insert(0, "/opt/trn_rl_repo")

import ml_dtypes

B, N, NE, EE = 1, 512, 256, 128
H, D = 8, 64
INNER = H * D
NCORES = 8
IB = N // NCORES          # query rows per core
JT = N // 128             # j tiles
G = 8                     # query rows per edge-DMA group
SCALE = float(D) ** -0.5

F32 = np.float32
BF16 = ml_dtypes.bfloat16

_PROG = None              # cached compiled Bass program


def _build():
    import concourse.bacc as bacc
    import concourse.tile as tile
    from concourse import mybir
    from concourse.masks import make_identity

    f32 = mybir.dt.float32
    f32r = mybir.dt.float32r
    bf16 = mybir.dt.bfloat16
    AF = mybir.ActivationFunctionType

    nc = bacc.Bacc("TRN2", target_bir_lowering=False, debug=False)

    # ---- DRAM I/O (per-core shapes; host pre-arranges layouts) ----
    d_nodes = nc.dram_tensor("nodes", [128, 4, NE], f32, kind="ExternalInput")
    d_qnodes = nc.dram_tensor("qnodes", [IB, NE], f32, kind="ExternalInput")
    d_egt = nc.dram_tensor("egt", [IB, EE, N], bf16, kind="ExternalInput")
    d_wq = nc.dram_tensor("wq", [128, 2, INNER], f32, kind="ExternalInput")
    d_wkv = nc.dram_tensor("wkv", [128, 2, 2 * INNER], f32r, kind="ExternalInput")
    d_we = nc.dram_tensor("we", [EE, INNER], f32, kind="ExternalInput")
    d_wo = nc.dram_tensor("wo", [128, 4, NE], f32r, kind="ExternalInput")
    d_bq = nc.dram_tensor("bq_s", [D, H], f32, kind="ExternalInput")
    d_cb = nc.dram_tensor("cb", [128, 4], f32, kind="ExternalInput")
    d_fb = nc.dram_tensor("fb", [1, NE], f32, kind="ExternalInput")
    d_out = nc.dram_tensor("out", [IB, NE], f32, kind="ExternalOutput")

    with tile.TileContext(nc) as tc:
        with (
            tc.tile_pool(name="consts", bufs=1) as consts,
            tc.tile_pool(name="persist", bufs=1) as persist,
            tc.tile_pool(name="eg", bufs=5) as egp,
            tc.tile_pool(name="post", bufs=4) as postp,
        ):
            # ---------------- load constants (SWDGE queue; HWDGE is
            # reserved for the big edge streams) ----------------
            wkv_sb = consts.tile([128, 2, 2 * INNER], f32r)
            nc.gpsimd.dma_start(out=wkv_sb[:], in_=d_wkv[:])
            nodes_sb = consts.tile([128, 4, NE], f32)
            nc.gpsimd.dma_start(out=nodes_sb[:], in_=d_nodes[:])
            qn_sb = consts.tile([IB, NE], f32)
            nc.gpsimd.dma_start(out=qn_sb[:], in_=d_qnodes[:])
            wq_sb = consts.tile([128, 2, INNER], f32)
            nc.gpsimd.dma_start(out=wq_sb[:], in_=d_wq[:])
            we_sb = consts.tile([EE, INNER], f32)
            nc.gpsimd.dma_start(out=we_sb[:], in_=d_we[:])
            wo_sb = consts.tile([128, 4, NE], f32r)
            nc.gpsimd.dma_start(out=wo_sb[:], in_=d_wo[:])
            bq_sb = consts.tile([D, H], f32)
            nc.gpsimd.dma_start(out=bq_sb[:], in_=d_bq[:])
            cb_sb = consts.tile([128, 4], f32)
            nc.gpsimd.dma_start(out=cb_sb[:], in_=d_cb[:])
            fb_sb = consts.tile([1, NE], f32)
            nc.gpsimd.dma_start(out=fb_sb[:], in_=d_fb[:])

            # edge stream: one HWDGE queue, issued up front; the [j, ee]
            # layout is derived on-chip by PE transposes
            egts = []
            for g in range(IB // G):
                egt = egp.tile([EE, G, N], bf16, tag="egt")
                nc.sync.dma_start(
                    out=egt[:],
                    in_=d_egt[g * G:(g + 1) * G].rearrange("g p j -> p g j"),
                )
                egts.append(egt)

            ident = consts.tile([128, 128], f32)
            make_identity(nc, ident[:])
            ident_bf = consts.tile([128, 128], bf16)
            make_identity(nc, ident_bf[:])
            ones1 = consts.tile([1, IB], f32)
            nc.vector.memset(ones1[:], 1.0)
            wq_bf = consts.tile([128, 2, INNER], bf16)
            nc.vector.tensor_copy(wq_bf[:], wq_sb[:])

            # ---------------- persistent intermediates ----------------
            nodesT = persist.tile([128, 2, N], f32r)         # [ne%128, c, n]
            qnT = persist.tile([128, 2, IB], bf16)           # [ne%128, c, i]
            kT = persist.tile([D, H, N], bf16)               # [d, h, j]
            v_sb = persist.tile([128, JT, H, D + 1], bf16)   # [j%128, jt, h, d|1]
            qT = persist.tile([D, H, IB], bf16)              # [d, h, i]
            weT = persist.tile([D, H, EE], bf16)             # [d, h, ee]
            qe_sb = persist.tile([EE, IB, H], bf16)          # [ee, i, h]
            sim1 = persist.tile([128, JT, H, IB], bf16)      # [j%128, jt, h, i]
            attnT = persist.tile([128, JT, H, IB], bf16)     # [j%128, jt, h, i]
            ae_sb = persist.tile([EE, H, IB], f32)           # [ee, h, i]
            oi_sb = persist.tile([IB, H, D], f32)            # [i, h, d]
            oiT = persist.tile([128, 4, IB], f32r)           # [inner%128, it, i]
            out_sb = persist.tile([IB, NE], f32)

            with (
                tc.tile_pool(name="psA", bufs=2, space="PSUM") as psA,
                tc.tile_pool(name="psB", bufs=2, space="PSUM") as psB,
            ):
                # nodesT: transpose nodes [n, ne] -> [ne, n]
                for t in range(4):
                    for c in range(2):
                        pt = psB.tile([128, 128], f32, tag="tr")
                        nc.tensor.transpose(
                            pt[:], nodes_sb[:, t, c * 128:(c + 1) * 128], ident[:]
                        )
                        nc.vector.tensor_copy(
                            nodesT[:, c, t * 128:(t + 1) * 128], pt[:]
                        )
                # qnT: transpose qnodes [i, ne] -> [ne, i] (cast bf16)
                for c in range(2):
                    pt = psB.tile([128, IB], f32, tag="tr")
                    nc.tensor.transpose(
                        pt[:], qn_sb[:, c * 128:(c + 1) * 128], ident[0:IB, 0:IB]
                    )
                    nc.vector.tensor_copy(qnT[:, c, :], pt[:])
                # weT: transpose We [ee, inner] -> per head [d, ee] (cast bf16)
                for h in range(H):
                    pt = psB.tile([D, EE], f32, tag="tr2")
                    nc.tensor.transpose(
                        pt[:], we_sb[:, h * D:(h + 1) * D], ident[:]
                    )
                    nc.vector.tensor_copy(weT[:, h, :], pt[:])

                # kT[h] = (Wkv_k^T @ nodesT) in f32r, stored bf16
                for h in range(H):
                    pk = psA.tile([D, N], f32, tag="big")
                    for c in range(2):
                        nc.tensor.matmul(
                            pk[:],
                            wkv_sb[:, c, h * D:(h + 1) * D],
                            nodesT[:, c, :],
                            start=(c == 0),
                            stop=(c == 1),
                        )
                    nc.vector.tensor_copy(kT[:, h, :], pk[:])

                # v[t] = nodes @ Wkv_v in f32r -> bf16 [jt,h,d] + ones col
                for t in range(JT):
                    pv = psA.tile([128, N], f32, tag="big")
                    for c in range(2):
                        nc.tensor.matmul(
                            pv[:],
                            nodesT[:, c, t * 128:(t + 1) * 128],
                            wkv_sb[:, c, INNER:2 * INNER],
                            start=(c == 0),
                            stop=(c == 1),
                        )
                    nc.vector.tensor_copy(v_sb[:, t, :, 0:D], pv[:])
                nc.vector.memset(v_sb[:, :, :, D:D + 1], 1.0)

                # qT[h] = scale * (Wq_h^T @ qnodesT + bq_h)  (bf16 matmul)
                for h in range(H):
                    pq = psB.tile([D, IB], f32, tag="trq")
                    for c in range(2):
                        nc.tensor.matmul(
                            pq[:],
                            wq_bf[:, c, h * D:(h + 1) * D],
                            qnT[:, c, :],
                            start=(c == 0),
                            stop=(c == 1),
                        )
                    nc.scalar.activation(
                        out=qT[:, h, :],
                        in_=pq[:],
                        func=AF.Identity,
                        bias=bq_sb[:, h:h + 1],
                        scale=SCALE,
                    )

                # qe[h] = WeT_h @ qT_h -> [ee, i]
                for h in range(H):
                    pqe = psB.tile([EE, IB], f32, tag="trq")
                    nc.tensor.matmul(
                        pqe[:], weT[:, h, :], qT[:, h, :], start=True, stop=True
                    )
                    nc.vector.tensor_copy(qe_sb[:, :, h], pqe[:])

                # sim1[jt,h] = kT_h[:, jt].T @ qT_h  (+ column mask bias)
                for h in range(H):
                    for jt in range(JT):
                        ps1 = psB.tile([128, IB], f32, tag="trq")
                        nc.tensor.matmul(
                            ps1[:],
                            kT[:, h, jt * 128:(jt + 1) * 128],
                            qT[:, h, :],
                            start=True,
                            stop=True,
                        )
                        nc.vector.tensor_scalar(
                            out=sim1[:, jt, h, :],
                            in0=ps1[:],
                            scalar1=cb_sb[:, jt:jt + 1],
                            scalar2=None,
                            op0=mybir.AluOpType.add,
                        )

            # ---------------- main loop over own query rows ----------------
            with (
                tc.tile_pool(name="egn", bufs=6) as egnp,
                tc.tile_pool(name="psS", bufs=3, space="PSUM") as psS,
                tc.tile_pool(name="psAE", bufs=2, space="PSUM") as psAE,
                tc.tile_pool(name="psT", bufs=3, space="PSUM") as psT,
            ):
                def tr_block(i):
                    "Derive the [j, ee] edge layout for row i via PE transpose."
                    gi, go = divmod(i, G)
                    pt = psT.tile([128, JT, EE], bf16, tag="ptr")
                    for jt in range(JT):
                        nc.tensor.transpose(
                            pt[:, jt, :],
                            egts[gi][:, go, jt * 128:(jt + 1) * 128],
                            ident_bf[:],
                        )
                    egn = egnp.tile([128, JT, EE], bf16, tag="egnd")
                    if i % 2 == 0:
                        nc.vector.tensor_copy(egn[:], pt[:])
                    else:
                        nc.scalar.copy(egn[:], pt[:])
                    return egn

                def sim_block(i, ps, u):
                    """sim1 copy + 4 sim2 matmuls into ps[:, u] for row i."""
                    gi, go = divmod(i, G)
                    nc.tensor.matmul(
                        ps[:, u, :, :],
                        ident_bf[:],
                        sim1[:, :, :, i],
                        start=True,
                        stop=False,
                        skip_group_check=True,
                    )
                    for jt in range(JT):
                        nc.tensor.matmul(
                            ps[:, u, jt, :],
                            egts[gi][:, go, jt * 128:(jt + 1) * 128],
                            qe_sb[:, i, :],
                            start=False,
                            stop=(jt == JT - 1),
                            skip_group_check=True,
                        )

                def ae_block(i, egn):
                    pae = psAE.tile([EE, H], f32, tag="ae")
                    for jt in range(JT):
                        nc.tensor.matmul(
                            pae[:],
                            egn[:, jt, :],
                            attnT[:, jt, :, i],
                            start=(jt == 0),
                            stop=(jt == JT - 1),
                        )
                    nc.vector.tensor_copy(ae_sb[:, :, i], pae[:])

                prev = None
                for p in range(IB // 2):
                    i0 = 2 * p
                    eg0 = tr_block(i0)
                    eg1 = tr_block(i0 + 1)
                    ps = psS.tile([128, 2, JT, H], f32, tag="sim")
                    sim_block(i0, ps, 0)
                    sim_block(i0 + 1, ps, 1)
                    nc.scalar.activation(
                        out=attnT[:, :, :, i0:i0 + 2].rearrange(
                            "p t h i -> p i t h"
                        ),
                        in_=ps[:],
                        func=AF.Exp,
                    )
                    if prev is not None:
                        ae_block(prev, peg0)
                        ae_block(prev + 1, peg1)
                    prev, peg0, peg1 = i0, eg0, eg1
                ae_block(prev, peg0)
                ae_block(prev + 1, peg1)

            # ---------------- epilogue ----------------
            with tc.tile_pool(name="psO", bufs=2, space="PSUM") as psO:
                for h in range(H):
                    po = psO.tile([IB, NE], f32, tag="po")
                    for jt in range(JT):
                        nc.tensor.matmul(
                            po[:, 0:D + 1],
                            attnT[:, jt, h, :],
                            v_sb[:, jt, h, :],
                            start=(jt == 0),
                            stop=False,
                            skip_group_check=True,
                        )
                    nc.tensor.matmul(
                        po[:, 0:D],
                        ae_sb[:, h, :],
                        we_sb[:, h * D:(h + 1) * D],
                        start=False,
                        stop=True,
                        skip_group_check=True,
                    )
                    rcp = postp.tile([IB, 1], f32, tag="rcp")
                    nc.vector.reciprocal(rcp[:], po[:, D:D + 1])
                    nc.vector.tensor_scalar_mul(oi_sb[:, h, :], po[:, 0:D], rcp[:])

                # transpose oi [i, inner] -> [inner, i]
                for it in range(4):
                    pt = psO.tile([128, IB], f32, tag="po")
                    nc.tensor.transpose(
                        pt[:],
                        oi_sb[:, it * 2:(it + 1) * 2, :],
                        ident[0:IB, 0:IB],
                    )
                    nc.vector.tensor_copy(oiT[:, it, :], pt[:])

                # out = oi @ Wo + final_bias
                pf = psO.tile([IB, NE], f32, tag="po")
                for it in range(4):
                    nc.tensor.matmul(
                        pf[:],
                        oiT[:, it, :],
                        wo_sb[:, it, :],
                        start=(it == 0),
                        stop=False,
                        skip_group_check=True,
                    )
                nc.tensor.matmul(
                    pf[:],
                    ones1[:],
                    fb_sb[:],
                    start=False,
                    stop=True,
                    skip_group_check=True,
                )
                nc.vector.tensor_copy(out_sb[:], pf[:])
                nc.gpsimd.dma_start(out=d_out[:], in_=out_sb[:])

    nc.compile()
    nc.finalize()
    return nc


def _get_prog():
    global _PROG
    if _PROG is None:
        _PROG = _build()
    return _PROG


def _prep_inputs(nodes, edges, mask, Wq, bq, Wkv, bkv, We, be, Wo, bo):
    """Host-side shard + layout prep. Returns list of 8 in_maps."""
    nodes = np.asarray(nodes, F32)[0]            # [N, NE]
    edges = np.asarray(edges, F32)[0]            # [N, N, EE]
    mask = np.asarray(mask)[0]                   # [N]
    Wq, bq = np.asarray(Wq, F32), np.asarray(bq, F32)
    Wkv, bkv = np.asarray(Wkv, F32), np.asarray(bkv, F32)
    We, be = np.asarray(We, F32), np.asarray(be, F32)
    Wo, bo = np.asarray(Wo, F32), np.asarray(bo, F32)

    nodes_pre = np.ascontiguousarray(
        nodes.reshape(4, 128, NE).transpose(1, 0, 2))          # [128, 4, NE]
    wq_pre = np.ascontiguousarray(
        Wq.reshape(2, 128, INNER).transpose(1, 0, 2))          # [128, 2, INNER]
    wkv_pre = np.ascontiguousarray(
        Wkv.reshape(2, 128, 2 * INNER).transpose(1, 0, 2))
    wo_pre = np.ascontiguousarray(
        Wo.reshape(4, 128, NE).transpose(1, 0, 2))             # [128, 4, NE]
    bq_pre = np.ascontiguousarray(
        (bq * SCALE).reshape(H, D).T)                          # [D, H]
    cb = np.where(mask, 0.0, -1e30).astype(F32)
    cb_pre = np.ascontiguousarray(cb.reshape(4, 128).T)        # [128, 4]
    fb = ((bkv[INNER:] + be) @ Wo + bo).astype(F32)[None, :]   # [1, NE]

    common = dict(
        nodes=nodes_pre, wq=wq_pre, wkv=wkv_pre, we=We, wo=wo_pre,
        bq_s=bq_pre, cb=cb_pre, fb=fb,
    )
    in_maps = []
    for c in range(NCORES):
        sl = edges[c * IB:(c + 1) * IB]                        # [IB, N, EE]
        egt = np.ascontiguousarray(sl.transpose(0, 2, 1)).astype(BF16)
        qn = np.ascontiguousarray(nodes[c * IB:(c + 1) * IB])
        in_maps.append(dict(common, qnodes=qn, egt=egt))
    return in_maps


def kernel(**inputs):
    from concourse.bass_utils import run_bass_kernel_spmd

    nc = _get_prog()
    in_maps = _prep_inputs(**inputs)
    res = run_bass_kernel_spmd(nc, in_maps, core_ids=list(range(NCORES)))
    out = np.concatenate([res.results[c]["out"] for c in range(NCORES)], axis=0)
    return out.reshape(B, N, NE).astype(F32)
